# revision 1
# baseline (speedup 1.0000x reference)
"""Distributed Trainium2 kernel for the dense transformer block.

Strategy (8 NeuronCores, SPMD):
  Phase A (token-parallel): each core owns 512 contiguous tokens (+3-token
    causal-conv halo). rmsnorm -> qkv matmul -> depthwise causal conv ->
    SiLU -> RoPE, all in feature-major layout (channels on partitions).
  AllToAll 1: reshard q/k/v from token-parallel to head-parallel.
  Phase B (head-parallel): each core runs causal flash-attention (no
    running max; scores are tiny for this problem) for its 2 heads over
    all 4096 tokens.
  AllToAll 2: reshard attention output y back to token-parallel.
  Phase C (token-parallel): proj + residual -> rmsnorm2 -> gated MLP ->
    residual. Output is feature-major (2048, 512) per core; the host
    reassembles (B, T, C).

All matmuls run with bf16 operands and f32 PSUM accumulation. Norm
scales, conv accumulation, residuals and softmax denominators stay f32.
"""
import os
import sys

sys.path.insert(0, "/opt/trn_rl_repo")

import numpy as np
import ml_dtypes

import concourse.bass as bass
import concourse.mybir as mybir
from concourse import bacc, tile
from concourse.bass_utils import run_bass_kernel_spmd

B, T, C = 2, 2048, 2048
NH, NG, HS = 16, 4, 128
QPK = NH // NG
DCONV = 4
IM = 5632
EPS = 1e-5
NCORES = 8
TOK = 512            # tokens per core
HALO = DCONV - 1
XW = TOK + HALO      # 515
CH = 259             # chunk width with halo (256 + 3)
NKC = C // 128       # 16
NMQ = (NH + 2 * NG)  # 24 qkv m-tiles
NMI = IM // 128      # 44
SCALE = 1.0 / float(np.sqrt(HS))

F32 = mybir.dt.float32
BF16 = mybir.dt.bfloat16
AF = mybir.ActivationFunctionType
ALU = mybir.AluOpType

DEBUG = bool(int(os.environ.get("KERNEL_DEBUG", "0")))
TRACE = bool(int(os.environ.get("KERNEL_TRACE", "0")))

LAST_RESULTS = None  # test.py reads exec_time from here


# --------------------------------------------------------------------------
# builder
# --------------------------------------------------------------------------

def build_nc():
    nc = bacc.Bacc("TRN2", target_bir_lowering=False, debug=False,
                   enable_asserts=True, num_devices=NCORES)

    x_d = nc.dram_tensor("x", [C, XW], F32, kind="ExternalInput")
    wq_d = nc.dram_tensor("wq", [NMQ, 128, C], BF16, kind="ExternalInput")
    wp_d = nc.dram_tensor("wp", [16, 128, C], BF16, kind="ExternalInput")
    w1_d = nc.dram_tensor("w1", [NMI, 128, C], BF16, kind="ExternalInput")
    w2_d = nc.dram_tensor("w2", [NMI, 128, C], BF16, kind="ExternalInput")
    wm_d = nc.dram_tensor("wm", [16, 128, IM], BF16, kind="ExternalInput")
    cw_d = nc.dram_tensor("cw", [128, NMQ * DCONV], F32, kind="ExternalInput")
    trig_d = nc.dram_tensor("trig", [128, 1024], BF16, kind="ExternalInput")
    msk_d = nc.dram_tensor("msk", [128, 2048], BF16, kind="ExternalInput")
    sel_d = nc.dram_tensor("sel", [8, 1024], BF16, kind="ExternalInput")
    rotm_d = nc.dram_tensor("rotm", [128, 128], BF16, kind="ExternalInput")
    out_d = nc.dram_tensor("out", [C, TOK], F32, kind="ExternalOutput")

    dbg = {}
    if DEBUG:
        dbg["sl"] = nc.dram_tensor("d_sl", [NMQ * 128, TOK], BF16, kind="ExternalOutput")
        dbg["t1o"] = nc.dram_tensor("d_t1o", [4096, 512], BF16, kind="ExternalOutput")
        dbg["y"] = nc.dram_tensor("d_y", [256, B * T], BF16, kind="ExternalOutput")
        dbg["x2"] = nc.dram_tensor("d_x2", [C, TOK], F32, kind="ExternalOutput")

    with tile.TileContext(nc) as tc:
        with tc.tile_pool(name="dram", bufs=1, space="DRAM") as dram, \
             tc.tile_pool(name="pers", bufs=1) as pers:
            t1i_kv = dram.tile([2048, 512], BF16)
            t1o_kv = dram.tile([2048, 512], BF16)
            t1i_q = dram.tile([2048, 512], BF16)
            t1o_q = dram.tile([2048, 512], BF16)
            t2i_a = dram.tile([1024, 512], BF16)
            t2o_a = dram.tile([1024, 512], BF16)
            t2i_b = dram.tile([1024, 512], BF16)
            t2o_b = dram.tile([1024, 512], BF16)

            # ---- constants ----
            cw_sb = pers.tile([128, NMQ * DCONV], F32, tag="cw", name="cw")
            trig_sb = pers.tile([128, 1024], BF16, tag="trig", name="trig")
            msk_sb = pers.tile([128, 2048], BF16, tag="msk", name="msk")
            sel_sb = pers.tile([8, 1024], BF16, tag="sel", name="sel")
            rotm = pers.tile([128, 128], BF16, tag="rotm", name="rotm")
            nc.sync.dma_start(cw_sb[:], cw_d[:])
            nc.sync.dma_start(trig_sb[:], trig_d[:])
            nc.sync.dma_start(msk_sb[:], msk_d[:])
            nc.sync.dma_start(sel_sb[:], sel_d[:])
            nc.sync.dma_start(rotm[:], rotm_d[:])

            ones128 = pers.tile([128, 128], BF16, tag="ones128", name="ones128")
            eps1 = pers.tile([1, 1], F32, tag="eps1", name="eps1")
            nc.gpsimd.memset(ones128[:], 1.0)
            nc.gpsimd.memset(eps1[:], EPS)

            # ---- persistent activations ----
            xh = [pers.tile([128, XW], F32, tag=f"xh{i}", name=f"xh{i}") for i in range(NKC)]
            yk = [pers.tile([128, TOK], BF16, tag=f"yk{i}", name=f"yk{i}")
                  for i in range(NKC)]
            for i in range(NKC):
                nc.sync.dma_start(xh[i][:], x_d[i * 128:(i + 1) * 128, :])

            # ============================================================
            # Phase A: norm1 -> qkv -> conv -> silu -> rope -> pack A2A1
            # ============================================================
            with tc.tile_pool(name="pa_sb", bufs=1) as pa, \
                 tc.tile_pool(name="pa_ps", bufs=1, space="PSUM") as pap:
                n1 = [pa.tile([128, 2, CH], BF16, tag=f"n1_{i}", name=f"n1_{i}")
                      for i in range(NKC)]
                for ch in range(2):
                    ss_ps = pap.tile([128, CH], F32, tag="ps1", bufs=3, name="ps1")
                    for kk in range(NKC):
                        xsq = pa.tile([128, CH], BF16, tag="xsq", bufs=3, name="xsq")
                        nc.scalar.activation(xsq[:], xh[kk][:, ch * 256:ch * 256 + CH],
                                             AF.Square)
                        nc.tensor.matmul(ss_ps[:], ones128[:], xsq[:],
                                         start=(kk == 0), stop=(kk == NKC - 1))
                    rt = pa.tile([1, CH], F32, tag="rt", bufs=2, name="rt")
                    nc.scalar.activation(rt[:], ss_ps[0:1, :], AF.Sqrt,
                                         bias=eps1[:], scale=1.0 / C)
                    rinv = pa.tile([1, CH], F32, tag="rinv", bufs=2, name="rinv")
                    nc.vector.reciprocal(rinv[:], rt[:])
                    rb_sb = pa.tile([128, CH], F32, tag="rb", bufs=2, name="rb")
                    nc.gpsimd.partition_broadcast(rb_sb[:], rinv[:])
                    for kk in range(NKC):
                        nc.vector.tensor_mul(n1[kk][:, ch, :],
                                             xh[kk][:, ch * 256:ch * 256 + CH],
                                             rb_sb[:])

                m_order = [g * 6 + sl for g in range(NG) for sl in (4, 5)] + \
                          [g * 6 + sl for g in range(NG) for sl in range(4)]
                for mi_, m in enumerate(m_order):
                    g, slot = m // 6, m % 6
                    wq_sb = pa.tile([128, C], BF16, tag="wq", bufs=2, name="wq")
                    nc.sync.dma_start(wq_sb[:], wq_d[m])
                    big = pap.tile([128, 1024], F32, tag="big", bufs=2, name="big")
                    for ch in range(2):
                        for kk in range(NKC):
                            nc.tensor.matmul(
                                big[:, ch * 512:ch * 512 + CH],
                                wq_sb[:, kk * 128:(kk + 1) * 128],
                                n1[kk][:, ch, :],
                                start=(kk == 0), stop=(kk == NKC - 1))
                    pre = pa.tile([128, 1024], BF16, tag="pre", bufs=2, name="pre")
                    nc.scalar.copy(pre[:], big[:])
                    src = pre[:].rearrange("p (c n) -> p c n", c=2)
                    acc = pa.tile([128, 2, 256], F32, tag="acc", bufs=2, name="acc")
                    nc.scalar.activation(acc[:], src[:, :, 0:256], AF.Copy,
                                         scale=cw_sb[:, m * 4:m * 4 + 1])
                    for j in range(1, DCONV):
                        nc.vector.scalar_tensor_tensor(
                            acc[:], src[:, :, j:j + 256],
                            cw_sb[:, m * 4 + j:m * 4 + j + 1], acc[:],
                            op0=ALU.mult, op1=ALU.add)
                    sl = pa.tile([128, 512], BF16, tag="sl", bufs=3, name="sl")
                    nc.scalar.activation(
                        sl[:].rearrange("p (c n) -> p c n", c=2), acc[:], AF.Silu)
                    if DEBUG:
                        nc.sync.dma_start(dbg["sl"][m * 128:(m + 1) * 128, :], sl[:])

                    if slot <= 4:  # q heads and k: rope
                        # rot = [-x2; x1] via PE rotation matmul, then
                        # ro = sl*[c;c] + rot*[s;s]
                        rot_ps = pap.tile([128, 512], F32, tag="ps1", bufs=3, name="ps1")
                        nc.tensor.matmul(rot_ps[:], rotm[:], sl[:],
                                         start=True, stop=True)
                        tt1 = pa.tile([128, 512], BF16, tag="tt1", bufs=2, name="tt1")
                        nc.vector.tensor_mul(tt1[:], sl[:], trig_sb[:, 0:512])
                        tt2 = pa.tile([128, 512], BF16, tag="tt2", bufs=2, name="tt2")
                        nc.vector.tensor_mul(tt2[:], rot_ps[:], trig_sb[:, 512:1024])
                        ro = pa.tile([128, 512], BF16, tag="ro", bufs=3, name="ro")
                        nc.vector.tensor_add(ro[:], tt1[:], tt2[:])
                        if slot < 4:
                            h = g * QPK + slot
                            nc.sync.dma_start(
                                t1i_q[(h // 2) * 256 + (h % 2) * 128:
                                      (h // 2) * 256 + (h % 2) * 128 + 128, :],
                                ro[:])
                        else:  # k -> both consumer cores
                            for d in (2 * g, 2 * g + 1):
                                nc.sync.dma_start(
                                    t1i_kv[d * 256:d * 256 + 128, :], ro[:])
                    else:  # v: transpose to token-major (DMA xbar transpose)
                        for i in range(4):
                            vts = pa.tile([128, 128], BF16, tag="vts", bufs=3, name="vts")
                            nc.sync.dma_start_transpose(vts[:], sl[:, i * 128:(i + 1) * 128])
                            for d in (2 * g, 2 * g + 1):
                                vreg = t1i_kv[d * 256 + 128:d * 256 + 256, :] \
                                    .rearrange("p (a b) -> (p a) b", b=128)
                                nc.sync.dma_start(
                                    vreg[i * 128:(i + 1) * 128, :], vts[:])
                    if mi_ == 7:  # all kv tiles written -> fire kv exchange
                        nc.gpsimd.collective_compute(
                            "AllToAll", ALU.bypass,
                            replica_groups=[list(range(NCORES))],
                            ins=[t1i_kv[:].opt()], outs=[t1o_kv[:].opt()])

            nc.gpsimd.collective_compute(
                "AllToAll", ALU.bypass,
                replica_groups=[list(range(NCORES))],
                ins=[t1i_q[:].opt()], outs=[t1o_q[:].opt()])
            if DEBUG:
                nc.sync.dma_start(dbg["t1o"][0:2048, :], t1o_kv[:])
                nc.sync.dma_start(dbg["t1o"][2048:4096, :], t1o_q[:])

            # ============================================================
            # Phase B: head-parallel causal attention (2 heads per core)
            # ============================================================
            with tc.tile_pool(name="pb_sb", bufs=1) as pb, \
                 tc.tile_pool(name="pb_ps", bufs=1, space="PSUM") as pbp:
                y_t = [pb.tile([128, B * T], BF16, tag=f"y{i}", name=f"y{i}")
                       for i in range(2)]
                for hl in range(2):
                    rho_raw = pb.tile([8, 512], F32, tag="rho_raw", bufs=2, name="rho_raw")
                    osb_all = {}
                    for beta in range(B):
                        kall = pb.tile([128, 2048], BF16, tag="kall", bufs=2, name="kall")
                        vall = pb.tile([128, 16, 128], BF16, tag="vall", bufs=2, name="vall")
                        for kb in range(8):
                            jj = beta * 4 + kb // 2
                            pos = kb % 2
                            nc.sync.dma_start(
                                kall[:, kb * 256:(kb + 1) * 256],
                                t1o_kv[jj * 256:jj * 256 + 128,
                                       pos * 256:(pos + 1) * 256])
                            vreg = t1o_kv[jj * 256 + 128:jj * 256 + 256, :] \
                                .rearrange("p (a b) -> (p a) b", b=128)
                            for i in range(2):
                                nc.sync.dma_start(
                                    vall[:, kb * 2 + i, :],
                                    vreg[pos * 256 + i * 128:pos * 256 + (i + 1) * 128, :])
                        qall = pb.tile([128, 2048], BF16, tag="qall", bufs=2, name="qall")
                        for bq in range(8):
                            jj = beta * 4 + bq // 2
                            pos = bq % 2
                            nc.sync.dma_start(
                                qall[:, bq * 256:(bq + 1) * 256],
                                t1o_q[jj * 256 + hl * 128:jj * 256 + (hl + 1) * 128,
                                      pos * 256:(pos + 1) * 256])
                        for bp in range(4):
                            o_ps = pbp.tile([128, 512], F32, tag="o", bufs=2, name="o")
                            rs_ps = pbp.tile([128, 512], F32, tag="rs", bufs=2, name="rs")
                            nkb = 2 * bp + 2
                            for kb in range(nkb):
                                s_ps = pbp.tile([128, 2, 512], F32, tag="s", bufs=2, name="s")
                                p_sb = pb.tile([128, 2, 512], BF16, tag="p", bufs=4, name="p")
                                # column offsets: skip fully-masked tq ranges in
                                # the two diagonal key blocks of each 512-pair
                                if kb == nkb - 2:
                                    c0s, mof = (0, 128), 0
                                elif kb == nkb - 1:
                                    c0s, mof = (256, 384), 1024
                                else:
                                    c0s, mof = (0, 0), None
                                for i in range(2):
                                    c0 = c0s[i]
                                    nc.tensor.matmul(
                                        s_ps[:, i, c0:],
                                        kall[:, kb * 256 + i * 128:kb * 256 + (i + 1) * 128],
                                        qall[:, bp * 512 + c0:(bp + 1) * 512],
                                        start=True, stop=True)
                                if mof is None:
                                    nc.scalar.activation(p_sb[:], s_ps[:], AF.Exp,
                                                         scale=SCALE)
                                else:
                                    for i in range(2):
                                        c0 = c0s[i]
                                        nc.scalar.activation(
                                            p_sb[:, i, c0:], s_ps[:, i, c0:],
                                            AF.Exp, scale=SCALE)
                                        nc.vector.tensor_mul(
                                            p_sb[:, i, c0:], p_sb[:, i, c0:],
                                            msk_sb[:, mof + i * 512 + c0:
                                                   mof + (i + 1) * 512])
                                for i in range(2):
                                    c0 = c0s[i]
                                    nc.tensor.matmul(
                                        o_ps[:, c0:], vall[:, kb * 2 + i, :],
                                        p_sb[:, i, c0:],
                                        start=(kb == 0 and i == 0),
                                        stop=(kb == nkb - 1 and i == 1))
                                    nc.tensor.matmul(
                                        rs_ps[:, c0:], ones128[:],
                                        p_sb[:, i, c0:],
                                        start=(kb == 0 and i == 0),
                                        stop=(kb == nkb - 1 and i == 1))
                            ot = pb.tile([128, 512], BF16, tag=f"osb{beta}_{bp}",
                                         bufs=1, name=f"osb{beta}_{bp}")
                            nc.scalar.copy(ot[:], o_ps[:])
                            osb_all[(beta, bp)] = ot
                            rsrow = pb.tile([1, 512], F32, tag="rsrow", bufs=3, name="rsrow")
                            nc.scalar.copy(rsrow[:], rs_ps[0:1, :])
                            nc.sync.dma_start(
                                rho_raw[beta * 4 + bp:beta * 4 + bp + 1, :], rsrow[:])
                    rho = pb.tile([8, 512], BF16, tag="rho", bufs=2, name="rho")
                    with nc.allow_low_precision(reason="softmax denom in bf16"):
                        nc.vector.reciprocal(rho[:], rho_raw[:])
                    for beta in range(B):
                        for bp in range(4):
                            r_ = beta * 4 + bp
                            rhob_ps = pbp.tile([128, 512], F32, tag="s", bufs=2, name="rhob")
                            nc.tensor.matmul(rhob_ps[:],
                                             sel_sb[:, r_ * 128:(r_ + 1) * 128],
                                             rho[:], start=True, stop=True)
                            nc.vector.tensor_mul(
                                y_t[hl][:, beta * 2048 + bp * 512:
                                        beta * 2048 + (bp + 1) * 512],
                                osb_all[(beta, bp)][:], rhob_ps[:])
                    # this head-half is complete: exchange it while the other
                    # half computes
                    t2ih = t2i_a if hl == 0 else t2i_b
                    t2oh = t2o_a if hl == 0 else t2o_b
                    for j in range(8):
                        nc.sync.dma_start(
                            t2ih[j * 128:(j + 1) * 128, :],
                            y_t[hl][:, j * 512:(j + 1) * 512])
                    nc.gpsimd.collective_compute(
                        "AllToAll", ALU.bypass,
                        replica_groups=[list(range(NCORES))],
                        ins=[t2ih[:].opt()], outs=[t2oh[:].opt()])
                if DEBUG:
                    for hl in range(2):
                        nc.sync.dma_start(dbg["y"][hl * 128:(hl + 1) * 128, :],
                                          y_t[hl][:])

            # ============================================================
            # Phase C: proj + residual, norm2, MLP, output
            # ============================================================
            with tc.tile_pool(name="pc_sb", bufs=1) as pc_, \
                 tc.tile_pool(name="pc_ps", bufs=1, space="PSUM") as pcp:
                x2 = [pc_.tile([128, TOK], F32, tag=f"x2_{i}", name=f"x2_{i}")
                      for i in range(NKC)]
                n2 = [pc_.tile([128, TOK], BF16, tag=f"n2_{i}", name=f"n2_{i}")
                      for i in range(NKC)]
                h_t = [pc_.tile([128, TOK], BF16, tag=f"h{i}", name=f"h{i}")
                       for i in range(NMI)]
                with tc.tile_pool(name="pcy", bufs=1) as pcy:
                    kk_order = list(range(0, NKC, 2)) + list(range(1, NKC, 2))
                    for kk in kk_order:
                        src = t2o_a if kk % 2 == 0 else t2o_b
                        nc.sync.dma_start(yk[kk][:],
                                          src[(kk // 2) * 128:(kk // 2 + 1) * 128, :])
                    evens = kk_order[:8]
                    odds = kk_order[8:]
                    for base in range(0, 16, 5):
                        blk = range(base, min(base + 5, 16))
                        mm_tiles = {}
                        wp_tiles = {}
                        for mo in blk:
                            wp_sb = pcy.tile([128, C], BF16, tag="wpst", bufs=6, name="wpst")
                            nc.sync.dma_start(wp_sb[:], wp_d[mo])
                            wp_tiles[mo] = wp_sb
                            mm_ps = pcp.tile([128, TOK], F32, tag="mm", bufs=6, name="mm")
                            mm_tiles[mo] = mm_ps
                            for ik, kk in enumerate(evens):
                                nc.tensor.matmul(mm_ps[:],
                                                 wp_sb[:, kk * 128:(kk + 1) * 128],
                                                 yk[kk][:],
                                                 start=(ik == 0), stop=False)
                        for mo in blk:
                            for ik, kk in enumerate(odds):
                                nc.tensor.matmul(mm_tiles[mo][:],
                                                 wp_tiles[mo][:, kk * 128:(kk + 1) * 128],
                                                 yk[kk][:],
                                                 start=False, stop=(ik == len(odds) - 1))
                            nc.vector.tensor_add(x2[mo][:], xh[mo][:, HALO:], mm_tiles[mo][:])
                            if DEBUG:
                                nc.sync.dma_start(dbg["x2"][mo * 128:(mo + 1) * 128, :],
                                                  x2[mo][:])

                ss2 = pcp.tile([128, TOK], F32, tag="nrm", bufs=2, name="nrm")
                for kk in range(NKC):
                    x2sq = pc_.tile([128, TOK], BF16, tag="x2sq", bufs=3, name="x2sq")
                    nc.scalar.activation(x2sq[:], x2[kk][:], AF.Square)
                    nc.tensor.matmul(ss2[:], ones128[:], x2sq[:],
                                     start=(kk == 0), stop=(kk == NKC - 1))
                rt2 = pc_.tile([1, TOK], F32, tag="rt2", bufs=1, name="rt2")
                nc.scalar.activation(rt2[:], ss2[0:1, :], AF.Sqrt, bias=eps1[:], scale=1.0 / C)
                rinv2 = pc_.tile([1, TOK], F32, tag="rinv2", bufs=1, name="rinv2")
                nc.vector.reciprocal(rinv2[:], rt2[:])
                rb2 = pc_.tile([128, TOK], F32, tag="rb2", bufs=1, name="rb2")
                nc.gpsimd.partition_broadcast(rb2[:], rinv2[:])
                for kk in range(NKC):
                    nc.vector.tensor_mul(n2[kk][:], x2[kk][:], rb2[:])

                for mi in range(NMI):
                    w1_sb = pc_.tile([128, C], BF16, tag="wst", bufs=3, name="wst")
                    nc.sync.dma_start(w1_sb[:], w1_d[mi])
                    h1_ps = pcp.tile([128, TOK], F32, tag="mm", bufs=6, name="mm")
                    for kk in range(NKC):
                        nc.tensor.matmul(h1_ps[:],
                                         w1_sb[:, kk * 128:(kk + 1) * 128],
                                         n2[kk][:],
                                         start=(kk == 0), stop=(kk == NKC - 1))
                    s1 = pc_.tile([128, TOK], BF16, tag="s1", bufs=2, name="s1")
                    nc.scalar.activation(s1[:], h1_ps[:], AF.Silu)
                    w2_sb = pc_.tile([128, C], BF16, tag="wst", bufs=3, name="wst")
                    nc.sync.dma_start(w2_sb[:], w2_d[mi])
                    h2_ps = pcp.tile([128, TOK], F32, tag="mm", bufs=6, name="mm")
                    for kk in range(NKC):
                        nc.tensor.matmul(h2_ps[:],
                                         w2_sb[:, kk * 128:(kk + 1) * 128],
                                         n2[kk][:],
                                         start=(kk == 0), stop=(kk == NKC - 1))
                    nc.vector.tensor_mul(h_t[mi][:], s1[:], h2_ps[:])

                with tc.tile_pool(name="pcm", bufs=1) as pcm:
                    for mo in range(16):
                        wm_sb = pcm.tile([128, IM], BF16, tag="wm", bufs=2, name="wm")
                        nc.sync.dma_start(wm_sb[:], wm_d[mo])
                        mp_ps = pcp.tile([128, TOK], F32, tag="mm", bufs=6, name="mm")
                        for ki in range(NMI):
                            nc.tensor.matmul(mp_ps[:],
                                             wm_sb[:, ki * 128:(ki + 1) * 128],
                                             h_t[ki][:],
                                             start=(ki == 0), stop=(ki == NMI - 1))
                        outsb = pc_.tile([128, TOK], F32, tag="outsb", bufs=2, name="outsb")
                        nc.vector.tensor_add(outsb[:], x2[mo][:], mp_ps[:])
                        nc.sync.dma_start(out_d[mo * 128:(mo + 1) * 128, :], outsb[:])

    nc.compile()
    return nc


# --------------------------------------------------------------------------
# host-side prep / gather
# --------------------------------------------------------------------------

def _prep_lhsT(w, nm, nk):
    """w: (out, in) f32 -> (nm, 128, nk*128) bf16 where
    prep[m][p][k*128+c] = w[m*128+c, k*128+p]."""
    o, i = w.shape
    assert o == nm * 128 and i == nk * 128
    r = w.reshape(nm, 128, nk, 128).transpose(0, 3, 2, 1)  # (m, p, k, c)
    return np.ascontiguousarray(r.reshape(nm, 128, nk * 128)).astype(ml_dtypes.bfloat16)


def _host_inputs(inputs):
    x = np.asarray(inputs["x"], np.float32)          # (B, T, C)
    cos = np.asarray(inputs["cos"], np.float32)      # (T, 64)
    sin = np.asarray(inputs["sin"], np.float32)
    n1w = np.asarray(inputs["norm1_w"], np.float32)
    n2w = np.asarray(inputs["norm2_w"], np.float32)

    # fold rmsnorm weights into the (pre-transposed) weight matrices
    attn_w = np.asarray(inputs["attn_w"], np.float32) * n1w[None, :]
    fc1_w = np.asarray(inputs["fc1_w"], np.float32) * n2w[None, :]
    fc2_w = np.asarray(inputs["fc2_w"], np.float32) * n2w[None, :]
    proj_w = np.asarray(inputs["proj_w"], np.float32)
    mlp_w = np.asarray(inputs["mlp_proj_w"], np.float32)

    wq = _prep_lhsT(attn_w, NMQ, NKC)
    wp = _prep_lhsT(proj_w, 16, NKC)
    w1 = _prep_lhsT(fc1_w, NMI, NKC)
    w2 = _prep_lhsT(fc2_w, NMI, NKC)
    wm = _prep_lhsT(mlp_w, 16, NMI)

    # conv weights in qkv m-tile order: per g: q0..q3 (qconv), k, v
    cw = np.zeros((NMQ, 128, DCONV), np.float32)
    qc = np.asarray(inputs["qconv_w"], np.float32)
    kc = np.asarray(inputs["kconv_w"], np.float32)
    vc = np.asarray(inputs["vconv_w"], np.float32)
    for g in range(NG):
        for s in range(QPK):
            cw[g * 6 + s] = qc[(g * QPK + s) * 128:(g * QPK + s + 1) * 128]
        cw[g * 6 + 4] = kc[g * 128:(g + 1) * 128]
        cw[g * 6 + 5] = vc[g * 128:(g + 1) * 128]
    cw = np.ascontiguousarray(cw.transpose(1, 0, 2).reshape(128, NMQ * DCONV))

    # paired-block diag masks, each (128, 2, 512) flattened to (128, 1024):
    # mskA: kb == nkb-2 (tk rel = i*128+p); mskB: kb == nkb-1 (tk rel = 256+i*128+p)
    p = np.arange(128)[:, None]
    f = np.arange(512)[None, :]
    mskA = np.concatenate([(p <= f), (p + 128 <= f)], axis=1)
    mskB = np.concatenate([(p + 256 <= f), (p + 384 <= f)], axis=1)
    msk = np.concatenate([mskA, mskB], axis=1).astype(np.float32)
    msk = msk.astype(ml_dtypes.bfloat16)

    # rho-broadcast selectors: sel[:, bq*128:(bq+1)*128] one-hot row bq
    sel = np.zeros((8, 1024), np.float32)
    for bq in range(8):
        sel[bq, bq * 128:(bq + 1) * 128] = 1.0
    sel = sel.astype(ml_dtypes.bfloat16)

    # rope rotation: rot = rotm.T @ x = [-x2; x1]
    rotm = np.zeros((128, 128), np.float32)
    for m in range(64):
        rotm[m + 64, m] = -1.0
        rotm[m, m + 64] = 1.0
    rotm = rotm.astype(ml_dtypes.bfloat16)

    # per-core x (feature-major with halo) and trig
    xt = x.transpose(0, 2, 1)                        # (B, C, T)
    xpad = np.concatenate([np.zeros((B, C, HALO), np.float32), xt], axis=2)
    cosT = cos.T                                     # (64, T)
    sinT = sin.T
    in_maps = []
    for c in range(NCORES):
        beta, tb = c // 4, (512 * c) % 2048
        xc = np.ascontiguousarray(xpad[beta, :, tb:tb + XW])
        cs = np.concatenate([cosT[:, tb:tb + TOK], cosT[:, tb:tb + TOK]], axis=0)
        ss = np.concatenate([sinT[:, tb:tb + TOK], sinT[:, tb:tb + TOK]], axis=0)
        trig = np.concatenate([cs, ss], axis=1).astype(ml_dtypes.bfloat16)
        in_maps.append({
            "x": xc, "wq": wq, "wp": wp, "w1": w1, "w2": w2, "wm": wm,
            "cw": cw, "trig": np.ascontiguousarray(trig), "msk": msk, "sel": sel,
            "rotm": rotm,
        })
    return in_maps


_NC_CACHE = None


def kernel(**inputs) -> np.ndarray:
    global LAST_RESULTS, _NC_CACHE
    if _NC_CACHE is None:
        _NC_CACHE = build_nc()
    nc = _NC_CACHE
    in_maps = _host_inputs(inputs)
    res = run_bass_kernel_spmd(nc, in_maps, list(range(NCORES)), trace=TRACE)
    LAST_RESULTS = res
    out = np.zeros((B, T, C), np.float32)
    for c in range(NCORES):
        oc = res.results[c]["out"]                   # (C, TOK) feature-major
        beta, tb = c // 4, (512 * c) % 2048
        out[beta, tb:tb + TOK, :] = oc.T
    return out



# revision 10
# speedup vs baseline: 1.1245x; 1.1245x over previous
"""Distributed Trainium2 kernel for the dense transformer block.

Strategy (8 NeuronCores, SPMD):
  Phase A (token-parallel): each core owns 512 contiguous tokens (+3-token
    causal-conv halo). rmsnorm -> qkv matmul -> depthwise causal conv ->
    SiLU -> RoPE, all in feature-major layout (channels on partitions).
  AllToAll 1a/1b/1c: reshard k/v then q (split by head parity so the
    first attention half can start while the second half exchanges).
  Phase B (head-parallel): each core runs causal attention (no running
    max; scores are bounded for this problem) for its 2 heads over all
    4096 tokens.
  AllToAll 2a/2b: reshard attention output y back to token-parallel,
    one exchange per head-half; proj's even-k half runs during the
    second exchange.
  Phase C (token-parallel): proj + residual -> rmsnorm2 -> gated MLP ->
    residual. Output is feature-major (2048, 512) per core; the host
    reassembles (B, T, C).

All matmuls run with bf16 operands and f32 PSUM accumulation. Norm
scales, conv accumulation, residuals and softmax denominators stay f32.
The softmax denominator uses the fact that a ones-lhsT matmul already
replicates the row-sum to all 128 partitions, so no broadcast is needed.
"""
import os
import sys

sys.path.insert(0, "/opt/trn_rl_repo")

import numpy as np
import ml_dtypes

import concourse.bass as bass
import concourse.mybir as mybir
from concourse import bacc, tile
from concourse.bass_utils import run_bass_kernel_spmd

B, T, C = 2, 2048, 2048
NH, NG, HS = 16, 4, 128
QPK = NH // NG
DCONV = 4
IM = 5632
EPS = 1e-5
NCORES = 8
TOK = 512            # tokens per core
HALO = DCONV - 1
XW = TOK + HALO      # 515
CH = 259             # qkv chunk width with halo (256 + 3)
NKC = C // 128       # 16
NMQ = (NH + 2 * NG)  # 24 qkv m-tiles
NMI = IM // 128      # 44
SCALE = 1.0 / float(np.sqrt(HS))

F32 = mybir.dt.float32
BF16 = mybir.dt.bfloat16
AF = mybir.ActivationFunctionType
ALU = mybir.AluOpType

DEBUG = bool(int(os.environ.get("KERNEL_DEBUG", "0")))
TRACE = bool(int(os.environ.get("KERNEL_TRACE", "0")))

LAST_RESULTS = None  # test.py reads exec_time from here


# --------------------------------------------------------------------------
# builder
# --------------------------------------------------------------------------

def build_nc():
    nc = bacc.Bacc("TRN2", target_bir_lowering=False, debug=False,
                   enable_asserts=True, num_devices=NCORES)

    x_d = nc.dram_tensor("x", [C, XW], F32, kind="ExternalInput")
    wq_d = nc.dram_tensor("wq", [NMQ, 128, C], BF16, kind="ExternalInput")
    wp_d = nc.dram_tensor("wp", [16, 128, C], BF16, kind="ExternalInput")
    w1_d = nc.dram_tensor("w1", [NMI, 128, C], BF16, kind="ExternalInput")
    w2_d = nc.dram_tensor("w2", [NMI, 128, C], BF16, kind="ExternalInput")
    wm_d = nc.dram_tensor("wm", [16, 128, IM], BF16, kind="ExternalInput")
    cw_d = nc.dram_tensor("cw", [128, NMQ * DCONV], F32, kind="ExternalInput")
    trig_d = nc.dram_tensor("trig", [128, 1024], BF16, kind="ExternalInput")
    msk_d = nc.dram_tensor("msk", [128, 2048], BF16, kind="ExternalInput")
    rotm_d = nc.dram_tensor("rotm", [128, 128], BF16, kind="ExternalInput")
    out_d = nc.dram_tensor("out", [C, TOK], F32, kind="ExternalOutput")

    dbg = {}
    if DEBUG:
        dbg["sl"] = nc.dram_tensor("d_sl", [NMQ * 128, TOK], BF16, kind="ExternalOutput")
        dbg["x2"] = nc.dram_tensor("d_x2", [C, TOK], F32, kind="ExternalOutput")

    with tile.TileContext(nc) as tc:
        with tc.tile_pool(name="dram", bufs=1, space="DRAM") as dram, \
             tc.tile_pool(name="pers", bufs=1) as pers:
            t1i_kv = dram.tile([2048, 512], BF16)
            t1o_kv = dram.tile([2048, 512], BF16)
            t1i_qa = dram.tile([1024, 512], BF16)
            t1o_qa = dram.tile([1024, 512], BF16)
            t1i_qb = dram.tile([1024, 512], BF16)
            t1o_qb = dram.tile([1024, 512], BF16)
            t2i_a = dram.tile([1024, 512], BF16)
            t2o_a = dram.tile([1024, 512], BF16)
            t2i_b = dram.tile([1024, 512], BF16)
            t2o_b = dram.tile([1024, 512], BF16)

            # ---- constants ----
            cw_sb = pers.tile([128, NMQ * DCONV], F32, tag="cw", name="cw")
            trig_sb = pers.tile([128, 1024], BF16, tag="trig", name="trig")
            msk_sb = pers.tile([128, 2048], BF16, tag="msk", name="msk")
            rotm = pers.tile([128, 128], BF16, tag="rotm", name="rotm")
            nc.sync.dma_start(cw_sb[:], cw_d[:])
            nc.sync.dma_start(trig_sb[:], trig_d[:])
            nc.sync.dma_start(msk_sb[:], msk_d[:])
            nc.sync.dma_start(rotm[:], rotm_d[:])

            ones128 = pers.tile([128, 128], BF16, tag="ones128", name="ones128")
            eps1 = pers.tile([128, 1], F32, tag="eps1", name="eps1")
            nc.gpsimd.memset(ones128[:], 1.0)
            nc.gpsimd.memset(eps1[:], EPS)

            # ---- persistent activations ----
            xh = [pers.tile([128, XW], F32, tag=f"xh{i}", name=f"xh{i}") for i in range(NKC)]
            for i in range(NKC):
                nc.sync.dma_start(xh[i][:], x_d[i * 128:(i + 1) * 128, :])

            # ============================================================
            # Phase A: norm1 -> qkv -> conv -> silu -> rope -> pack A2A1
            # ============================================================
            with tc.tile_pool(name="pa_sb", bufs=1) as pa, \
                 tc.tile_pool(name="pa_ps", bufs=1, space="PSUM") as pap:
                n1 = [pa.tile([128, 2, CH], BF16, tag=f"n1_{i}", name=f"n1_{i}")
                      for i in range(NKC)]
                # --- rmsnorm1: squares (ACT), ones-matmul reduce, full-
                # partition sqrt + reciprocal (no broadcast needed) ---
                ss_ps = pap.tile([128, 2, 512], F32, tag="ss", bufs=1, name="ss")
                for kk in range(NKC):
                    t = pa.tile([128, 2, CH], BF16, tag="xsq", bufs=4, name="xsq")
                    for ch in range(2):
                        nc.scalar.activation(t[:, ch, :],
                                             xh[kk][:, ch * 256:ch * 256 + CH],
                                             AF.Square)
                    for ch in range(2):
                        nc.tensor.matmul(ss_ps[:, ch, 0:CH], ones128[:], t[:, ch, :],
                                         start=(kk == 0), stop=(kk == NKC - 1))
                rt = pa.tile([128, 2, CH], F32, tag="rt", bufs=1, name="rt")
                for ch in range(2):
                    nc.scalar.activation(rt[:, ch, :], ss_ps[:, ch, 0:CH], AF.Sqrt,
                                         bias=eps1[:], scale=1.0 / C)
                rinv = pa.tile([128, 2, CH], F32, tag="rinv", bufs=1, name="rinv")
                for ch in range(2):
                    nc.vector.reciprocal_approx_fast(rinv[:, ch, :], rt[:, ch, :])
                for kk in range(NKC):
                    for ch in range(2):
                        nc.vector.tensor_mul(n1[kk][:, ch, :],
                                             xh[kk][:, ch * 256:ch * 256 + CH],
                                             rinv[:, ch, :])

                # --- qkv m-tiles: kv first, then even-parity q heads, then
                # odd-parity q heads; A2As fire as each group completes ---
                m_order = [g * 6 + sl for g in range(NG) for sl in (4, 5)] + \
                          [g * 6 + sl for sl in (0, 2) for g in range(NG)] + \
                          [g * 6 + sl for sl in (1, 3) for g in range(NG)]
                for mi_, m in enumerate(m_order):
                    g, slot = m // 6, m % 6
                    wq_sb = pa.tile([128, C], BF16, tag="wq", bufs=3, name="wq")
                    nc.sync.dma_start(wq_sb[:], wq_d[m])
                    big = pap.tile([128, 1024], F32, tag="big", bufs=2, name="big")
                    for kk in range(NKC):
                        for ch in range(2):
                            nc.tensor.matmul(
                                big[:, ch * 512:ch * 512 + CH],
                                wq_sb[:, kk * 128:(kk + 1) * 128],
                                n1[kk][:, ch, :],
                                start=(kk == 0), stop=(kk == NKC - 1))
                    # stitch the two halo-overlapped chunks into one
                    # contiguous 515-wide bf16 strip for the conv taps
                    pre = pa.tile([128, XW], BF16, tag="pre", bufs=3, name="pre")
                    nc.scalar.copy(pre[:, 0:CH], big[:, 0:CH])
                    nc.scalar.copy(pre[:, CH:XW], big[:, 512 + HALO:512 + CH])
                    acc = pa.tile([128, 512], F32, tag="acc", bufs=2, name="acc")
                    nc.vector.tensor_scalar_mul(acc[:], pre[:, 0:512],
                                                cw_sb[:, m * 4:m * 4 + 1])
                    for j in range(1, DCONV):
                        nc.vector.scalar_tensor_tensor(
                            acc[:], pre[:, j:j + 512],
                            cw_sb[:, m * 4 + j:m * 4 + j + 1], acc[:],
                            op0=ALU.mult, op1=ALU.add)
                    sl = pa.tile([128, 512], BF16, tag="sl", bufs=3, name="sl")
                    nc.scalar.activation(sl[:], acc[:], AF.Silu)
                    if DEBUG:
                        nc.sync.dma_start(dbg["sl"][m * 128:(m + 1) * 128, :], sl[:])

                    if slot <= 4:  # q heads and k: rope
                        # rot = [-x2; x1] via PE rotation matmul, then
                        # ro = sl*[c;c] + rot*[s;s]
                        rot_ps = pap.tile([128, 512], F32, tag="rot", bufs=2, name="rot")
                        nc.tensor.matmul(rot_ps[:], rotm[:], sl[:],
                                         start=True, stop=True)
                        tt1 = pa.tile([128, 512], BF16, tag="tt1", bufs=2, name="tt1")
                        nc.vector.tensor_mul(tt1[:], sl[:], trig_sb[:, 0:512])
                        tt2 = pa.tile([128, 512], BF16, tag="tt2", bufs=2, name="tt2")
                        nc.vector.tensor_mul(tt2[:], rot_ps[:], trig_sb[:, 512:1024])
                        ro = pa.tile([128, 512], BF16, tag="ro", bufs=3, name="ro")
                        nc.vector.tensor_add(ro[:], tt1[:], tt2[:])
                        if slot < 4:
                            h = g * QPK + slot
                            dst = t1i_qa if slot % 2 == 0 else t1i_qb
                            d = h // 2
                            nc.sync.dma_start(dst[d * 128:(d + 1) * 128, :], ro[:])
                        else:  # k -> both consumer cores
                            for d in (2 * g, 2 * g + 1):
                                nc.sync.dma_start(
                                    t1i_kv[d * 256:d * 256 + 128, :], ro[:])
                    else:  # v: transpose to token-major (DMA xbar transpose)
                        vtb = pa.tile([128, 512], BF16, tag="vtb", bufs=2, name="vtb")
                        for i in range(4):
                            nc.sync.dma_start_transpose(
                                vtb[:, i * 128:(i + 1) * 128],
                                sl[:, i * 128:(i + 1) * 128])
                        for d in (2 * g, 2 * g + 1):
                            nc.sync.dma_start(
                                t1i_kv[d * 256 + 128:d * 256 + 256, :], vtb[:])
                    if mi_ == 7:  # all kv tiles written -> fire kv exchange
                        nc.gpsimd.collective_compute(
                            "AllToAll", ALU.bypass,
                            replica_groups=[list(range(NCORES))],
                            ins=[t1i_kv[:].opt()], outs=[t1o_kv[:].opt()])
                    if mi_ == 15:  # even-parity q heads -> first q exchange
                        nc.gpsimd.collective_compute(
                            "AllToAll", ALU.bypass,
                            replica_groups=[list(range(NCORES))],
                            ins=[t1i_qa[:].opt()], outs=[t1o_qa[:].opt()])

            nc.gpsimd.collective_compute(
                "AllToAll", ALU.bypass,
                replica_groups=[list(range(NCORES))],
                ins=[t1i_qb[:].opt()], outs=[t1o_qb[:].opt()])

            # ============================================================
            # Phase B: head-parallel causal attention (2 heads per core)
            # + B->C overlap: proj even-k half during the second y A2A
            # ============================================================
            with tc.tile_pool(name="bc_sb", bufs=1) as bc:
                yk_all = bc.tile([128, 16, 512], BF16, tag="yk", name="yk")
                x2 = [bc.tile([128, TOK], F32, tag=f"x2_{i}", name=f"x2_{i}")
                      for i in range(NKC)]
                kv_src = t1o_kv[:].rearrange("(j r) c -> r j c", r=256)
                with tc.tile_pool(name="pb_sb", bufs=1) as pb, \
                     tc.tile_pool(name="pb_ps", bufs=1, space="PSUM") as pbp:
                    y_t = [pb.tile([128, B * T], BF16, tag=f"y{i}", name=f"y{i}")
                           for i in range(2)]
                    for hl in range(2):
                        qsrcT = (t1o_qa if hl == 0 else t1o_qb)
                        q_src = qsrcT[:].rearrange("(j r) c -> r j c", r=128)
                        for beta in range(B):
                            kall = pb.tile([128, 2048], BF16, tag="kall", bufs=2, name="kall")
                            nc.sync.dma_start(
                                kall[:].rearrange("p (j c) -> p j c", c=512),
                                kv_src[0:128, beta * 4:beta * 4 + 4, :])
                            vall = pb.tile([128, 16, 128], BF16, tag="vall", bufs=2, name="vall")
                            nc.sync.dma_start(
                                vall[:].rearrange("p (j i) h -> p j (i h)", j=4),
                                kv_src[128:256, beta * 4:beta * 4 + 4, :])
                            qall = pb.tile([128, 2048], BF16, tag="qall", bufs=2, name="qall")
                            nc.sync.dma_start(
                                qall[:].rearrange("p (j c) -> p j c", c=512),
                                q_src[:, beta * 4:beta * 4 + 4, :])
                            for bp in range(4):
                                o_ps = pbp.tile([128, 512], F32, tag="o", bufs=2, name="o")
                                rs_ps = pbp.tile([128, 512], F32, tag="rs", bufs=2, name="rs")
                                nkb = 2 * bp + 2
                                for kb in range(nkb):
                                    s_ps = pbp.tile([128, 2, 512], F32, tag="s", bufs=2, name="s")
                                    p_sb = pb.tile([128, 2, 512], BF16, tag="p", bufs=4, name="p")
                                    # column offsets: skip fully-masked tq
                                    # ranges in the two diagonal key blocks
                                    if kb == nkb - 2:
                                        c0s, mof = (0, 128), 0
                                    elif kb == nkb - 1:
                                        c0s, mof = (256, 384), 1024
                                    else:
                                        c0s, mof = (0, 0), None
                                    for i in range(2):
                                        c0 = c0s[i]
                                        nc.tensor.matmul(
                                            s_ps[:, i, c0:],
                                            kall[:, kb * 256 + i * 128:kb * 256 + (i + 1) * 128],
                                            qall[:, bp * 512 + c0:(bp + 1) * 512],
                                            start=True, stop=True)
                                    if mof is None:
                                        nc.scalar.activation(p_sb[:], s_ps[:], AF.Exp,
                                                             scale=SCALE)
                                    else:
                                        for i in range(2):
                                            c0 = c0s[i]
                                            nc.scalar.activation(
                                                p_sb[:, i, c0:], s_ps[:, i, c0:],
                                                AF.Exp, scale=SCALE)
                                            nc.vector.tensor_mul(
                                                p_sb[:, i, c0:], p_sb[:, i, c0:],
                                                msk_sb[:, mof + i * 512 + c0:
                                                       mof + (i + 1) * 512])
                                    for i in range(2):
                                        c0 = c0s[i]
                                        nc.tensor.matmul(
                                            o_ps[:, c0:], vall[:, kb * 2 + i, :],
                                            p_sb[:, i, c0:],
                                            start=(kb == 0 and i == 0),
                                            stop=(kb == nkb - 1 and i == 1))
                                    for i in range(2):
                                        c0 = c0s[i]
                                        nc.tensor.matmul(
                                            rs_ps[:, c0:], ones128[:],
                                            p_sb[:, i, c0:],
                                            start=(kb == 0 and i == 0),
                                            stop=(kb == nkb - 1 and i == 1))
                                # the ones-lhsT matmul replicated the softmax
                                # denominator to every partition: reciprocal +
                                # multiply directly, no gather/broadcast.
                                recip = pb.tile([128, 512], F32, tag="recip", bufs=2, name="recip")
                                nc.vector.reciprocal_approx_fast(recip[:], rs_ps[:])
                                nc.vector.tensor_mul(
                                    y_t[hl][:, beta * 2048 + bp * 512:
                                            beta * 2048 + (bp + 1) * 512],
                                    o_ps[:], recip[:])
                        # this head-half is complete: exchange it while the
                        # other half computes
                        t2ih = t2i_a if hl == 0 else t2i_b
                        t2oh = t2o_a if hl == 0 else t2o_b
                        nc.sync.dma_start(
                            t2ih[:].rearrange("(j r) c -> r j c", r=128),
                            y_t[hl][:].rearrange("p (j c) -> p j c", c=512))
                        nc.gpsimd.collective_compute(
                            "AllToAll", ALU.bypass,
                            replica_groups=[list(range(NCORES))],
                            ins=[t2ih[:].opt()], outs=[t2oh[:].opt()])
                        yk_par = yk_all[:].rearrange("p (k par) c -> p k par c",
                                                     par=2)
                        nc.sync.dma_start(
                            yk_par[:, :, hl, :],
                            t2oh[:].rearrange("(j r) c -> r j c", r=128))

                    # ---- proj even-k half: only needs t2o_a; fills the PE
                    # while the second y exchange is in flight ----
                    with tc.tile_pool(name="pj_sb", bufs=1) as pj:
                        for mo in range(16):
                            wp_sb = pj.tile([128, C], BF16, tag="wpst", bufs=4, name="wpst")
                            nc.sync.dma_start(wp_sb[:], wp_d[mo])
                            mm_ps = pbp.tile([128, 512], F32, tag="s", bufs=2, name="pmm")
                            for ik in range(8):
                                kk = 2 * ik
                                nc.tensor.matmul(mm_ps[:],
                                                 wp_sb[:, kk * 128:(kk + 1) * 128],
                                                 yk_all[:, kk, :],
                                                 start=(ik == 0), stop=(ik == 7))
                            nc.vector.tensor_add(x2[mo][:], xh[mo][:, HALO:], mm_ps[:])

                # ---- proj odd-k half + norm2 + MLP ----
                with tc.tile_pool(name="pc_sb", bufs=1) as pc_, \
                     tc.tile_pool(name="pc_ps", bufs=1, space="PSUM") as pcp:
                    for mo in range(16):
                        wp_sb = pc_.tile([128, C], BF16, tag="wst", bufs=4, name="wst")
                        nc.sync.dma_start(wp_sb[:], wp_d[mo])
                        mm_ps = pcp.tile([128, 512], F32, tag="mm", bufs=6, name="mm")
                        for ik in range(8):
                            kk = 2 * ik + 1
                            nc.tensor.matmul(mm_ps[:],
                                             wp_sb[:, kk * 128:(kk + 1) * 128],
                                             yk_all[:, kk, :],
                                             start=(ik == 0), stop=(ik == 7))
                        nc.vector.tensor_add(x2[mo][:], x2[mo][:], mm_ps[:])
                        if DEBUG:
                            nc.sync.dma_start(dbg["x2"][mo * 128:(mo + 1) * 128, :],
                                              x2[mo][:])

                    n2 = [pc_.tile([128, TOK], BF16, tag=f"n2_{i}", name=f"n2_{i}")
                          for i in range(NKC)]
                    h_t = [pc_.tile([128, TOK], BF16, tag=f"h{i}", name=f"h{i}")
                           for i in range(NMI)]
                    ss2 = pcp.tile([128, TOK], F32, tag="nrm", bufs=1, name="nrm")
                    for kk in range(NKC):
                        t = pc_.tile([128, TOK], BF16, tag="x2sq", bufs=4, name="x2sq")
                        nc.scalar.activation(t[:], x2[kk][:], AF.Square)
                        nc.tensor.matmul(ss2[:], ones128[:], t[:],
                                         start=(kk == 0), stop=(kk == NKC - 1))
                    rt2 = pc_.tile([128, TOK], F32, tag="rt2", bufs=1, name="rt2")
                    nc.scalar.activation(rt2[:], ss2[:], AF.Sqrt, bias=eps1[:],
                                         scale=1.0 / C)
                    rinv2 = pc_.tile([128, TOK], F32, tag="rinv2", bufs=1, name="rinv2")
                    nc.vector.reciprocal_approx_fast(rinv2[:], rt2[:])
                    for kk in range(NKC):
                        nc.vector.tensor_mul(n2[kk][:], x2[kk][:], rinv2[:])

                    for mi in range(NMI):
                        w1_sb = pc_.tile([128, C], BF16, tag="wst", bufs=4, name="wst")
                        nc.sync.dma_start(w1_sb[:], w1_d[mi])
                        h1_ps = pcp.tile([128, TOK], F32, tag="mm", bufs=6, name="mm")
                        for kk in range(NKC):
                            nc.tensor.matmul(h1_ps[:],
                                             w1_sb[:, kk * 128:(kk + 1) * 128],
                                             n2[kk][:],
                                             start=(kk == 0), stop=(kk == NKC - 1))
                        s1 = pc_.tile([128, TOK], BF16, tag="s1", bufs=2, name="s1")
                        nc.scalar.activation(s1[:], h1_ps[:], AF.Silu)
                        w2_sb = pc_.tile([128, C], BF16, tag="wst", bufs=4, name="wst")
                        nc.sync.dma_start(w2_sb[:], w2_d[mi])
                        h2_ps = pcp.tile([128, TOK], F32, tag="mm", bufs=6, name="mm")
                        for kk in range(NKC):
                            nc.tensor.matmul(h2_ps[:],
                                             w2_sb[:, kk * 128:(kk + 1) * 128],
                                             n2[kk][:],
                                             start=(kk == 0), stop=(kk == NKC - 1))
                        nc.vector.tensor_mul(h_t[mi][:], s1[:], h2_ps[:])

                    with tc.tile_pool(name="pcm", bufs=1) as pcm:
                        for mo in range(16):
                            wm_sb = pcm.tile([128, IM], BF16, tag="wm", bufs=2, name="wm")
                            nc.sync.dma_start(wm_sb[:], wm_d[mo])
                            mp_ps = pcp.tile([128, TOK], F32, tag="mm", bufs=6, name="mm")
                            for ki in range(NMI):
                                nc.tensor.matmul(mp_ps[:],
                                                 wm_sb[:, ki * 128:(ki + 1) * 128],
                                                 h_t[ki][:],
                                                 start=(ki == 0), stop=(ki == NMI - 1))
                            outsb = pc_.tile([128, TOK], F32, tag="outsb", bufs=2, name="outsb")
                            nc.vector.tensor_add(outsb[:], x2[mo][:], mp_ps[:])
                            nc.sync.dma_start(out_d[mo * 128:(mo + 1) * 128, :], outsb[:])

    nc.compile()
    return nc


# --------------------------------------------------------------------------
# host-side prep / gather
# --------------------------------------------------------------------------

def _prep_lhsT(w, nm, nk):
    """w: (out, in) f32 -> (nm, 128, nk*128) bf16 where
    prep[m][p][k*128+c] = w[m*128+c, k*128+p]."""
    o, i = w.shape
    assert o == nm * 128 and i == nk * 128
    r = w.reshape(nm, 128, nk, 128).transpose(0, 3, 2, 1)  # (m, p, k, c)
    return np.ascontiguousarray(r.reshape(nm, 128, nk * 128)).astype(ml_dtypes.bfloat16)


def _host_inputs(inputs):
    x = np.asarray(inputs["x"], np.float32)          # (B, T, C)
    cos = np.asarray(inputs["cos"], np.float32)      # (T, 64)
    sin = np.asarray(inputs["sin"], np.float32)
    n1w = np.asarray(inputs["norm1_w"], np.float32)
    n2w = np.asarray(inputs["norm2_w"], np.float32)

    # fold rmsnorm weights into the (pre-transposed) weight matrices
    attn_w = np.asarray(inputs["attn_w"], np.float32) * n1w[None, :]
    fc1_w = np.asarray(inputs["fc1_w"], np.float32) * n2w[None, :]
    fc2_w = np.asarray(inputs["fc2_w"], np.float32) * n2w[None, :]
    proj_w = np.asarray(inputs["proj_w"], np.float32)
    mlp_w = np.asarray(inputs["mlp_proj_w"], np.float32)

    wq = _prep_lhsT(attn_w, NMQ, NKC)
    wp = _prep_lhsT(proj_w, 16, NKC)
    w1 = _prep_lhsT(fc1_w, NMI, NKC)
    w2 = _prep_lhsT(fc2_w, NMI, NKC)
    wm = _prep_lhsT(mlp_w, 16, NMI)

    # conv weights in qkv m-tile order: per g: q0..q3 (qconv), k, v
    cw = np.zeros((NMQ, 128, DCONV), np.float32)
    qc = np.asarray(inputs["qconv_w"], np.float32)
    kc = np.asarray(inputs["kconv_w"], np.float32)
    vc = np.asarray(inputs["vconv_w"], np.float32)
    for g in range(NG):
        for s in range(QPK):
            cw[g * 6 + s] = qc[(g * QPK + s) * 128:(g * QPK + s + 1) * 128]
        cw[g * 6 + 4] = kc[g * 128:(g + 1) * 128]
        cw[g * 6 + 5] = vc[g * 128:(g + 1) * 128]
    cw = np.ascontiguousarray(cw.transpose(1, 0, 2).reshape(128, NMQ * DCONV))

    # paired-block diag masks, each (128, 2, 512) flattened to (128, 1024):
    # mskA: kb == nkb-2 (tk rel = i*128+p); mskB: kb == nkb-1 (tk rel = 256+i*128+p)
    p = np.arange(128)[:, None]
    f = np.arange(512)[None, :]
    mskA = np.concatenate([(p <= f), (p + 128 <= f)], axis=1)
    mskB = np.concatenate([(p + 256 <= f), (p + 384 <= f)], axis=1)
    msk = np.concatenate([mskA, mskB], axis=1).astype(np.float32)
    msk = msk.astype(ml_dtypes.bfloat16)

    # rope rotation: rot = rotm.T @ x = [-x2; x1]
    rotm = np.zeros((128, 128), np.float32)
    for m in range(64):
        rotm[m + 64, m] = -1.0
        rotm[m, m + 64] = 1.0
    rotm = rotm.astype(ml_dtypes.bfloat16)

    # per-core x (feature-major with halo) and trig
    xt = x.transpose(0, 2, 1)                        # (B, C, T)
    xpad = np.concatenate([np.zeros((B, C, HALO), np.float32), xt], axis=2)
    cosT = cos.T                                     # (64, T)
    sinT = sin.T
    in_maps = []
    for c in range(NCORES):
        beta, tb = c // 4, (512 * c) % 2048
        xc = np.ascontiguousarray(xpad[beta, :, tb:tb + XW])
        cs = np.concatenate([cosT[:, tb:tb + TOK], cosT[:, tb:tb + TOK]], axis=0)
        ss = np.concatenate([sinT[:, tb:tb + TOK], sinT[:, tb:tb + TOK]], axis=0)
        trig = np.concatenate([cs, ss], axis=1).astype(ml_dtypes.bfloat16)
        in_maps.append({
            "x": xc, "wq": wq, "wp": wp, "w1": w1, "w2": w2, "wm": wm,
            "cw": cw, "trig": np.ascontiguousarray(trig), "msk": msk,
            "rotm": rotm,
        })
    return in_maps


_NC_CACHE = None


def kernel(**inputs) -> np.ndarray:
    global LAST_RESULTS, _NC_CACHE
    if _NC_CACHE is None:
        _NC_CACHE = build_nc()
    nc = _NC_CACHE
    in_maps = _host_inputs(inputs)
    res = run_bass_kernel_spmd(nc, in_maps, list(range(NCORES)), trace=TRACE)
    LAST_RESULTS = res
    out = np.zeros((B, T, C), np.float32)
    for c in range(NCORES):
        oc = res.results[c]["out"]                   # (C, TOK) feature-major
        beta, tb = c // 4, (512 * c) % 2048
        out[beta, tb:tb + TOK, :] = oc.T
    return out


# revision 17
# speedup vs baseline: 1.1712x; 1.0415x over previous
"""Distributed Trainium2 kernel for the dense transformer block.

Strategy (8 NeuronCores, SPMD):
  Phase A (token-parallel): each core owns 512 contiguous tokens (+3-token
    causal-conv halo). rmsnorm -> qkv matmul -> depthwise causal conv ->
    SiLU -> RoPE, all in feature-major layout (channels on partitions).
  AllToAll 1a/1b/1c: reshard k/v then q (split by head parity so the
    first attention half can start while the second half exchanges).
  Phase B (head-parallel): each core runs causal attention (no running
    max; scores are bounded for this problem) for its 2 heads over all
    4096 tokens.
  AllToAll 2a/2b: reshard attention output y back to token-parallel,
    one exchange per head-half; proj's even-k half runs during the
    second exchange.
  Phase C (token-parallel): proj + residual -> rmsnorm2 -> gated MLP ->
    residual. Output is feature-major (2048, 512) per core; the host
    reassembles (B, T, C).

All matmuls run with bf16 operands and f32 PSUM accumulation. Norm
scales, conv accumulation, residuals and softmax denominators stay f32.
The softmax denominator uses the fact that a ones-lhsT matmul already
replicates the row-sum to all 128 partitions, so no broadcast is needed.
"""
import os
import sys

sys.path.insert(0, "/opt/trn_rl_repo")

import numpy as np
import ml_dtypes

import concourse.bass as bass
import concourse.mybir as mybir
from concourse import bacc, tile
from concourse.bass_utils import run_bass_kernel_spmd

B, T, C = 2, 2048, 2048
NH, NG, HS = 16, 4, 128
QPK = NH // NG
DCONV = 4
IM = 5632
EPS = 1e-5
NCORES = 8
TOK = 512            # tokens per core
HALO = DCONV - 1
XW = TOK + HALO      # 515
CH = 259             # qkv chunk width with halo (256 + 3)
NKC = C // 128       # 16
NMQ = (NH + 2 * NG)  # 24 qkv m-tiles
NMI = IM // 128      # 44
SCALE = 1.0 / float(np.sqrt(HS))

F32 = mybir.dt.float32
BF16 = mybir.dt.bfloat16
AF = mybir.ActivationFunctionType
ALU = mybir.AluOpType

DEBUG = bool(int(os.environ.get("KERNEL_DEBUG", "0")))
TRACE = bool(int(os.environ.get("KERNEL_TRACE", "0")))

LAST_RESULTS = None  # test.py reads exec_time from here


# --------------------------------------------------------------------------
# builder
# --------------------------------------------------------------------------

def build_nc():
    nc = bacc.Bacc("TRN2", target_bir_lowering=False, debug=False,
                   enable_asserts=True, num_devices=NCORES)

    x_d = nc.dram_tensor("x", [C, XW], F32, kind="ExternalInput")
    wq_d = nc.dram_tensor("wq", [NMQ, 128, C], BF16, kind="ExternalInput")
    wp_d = nc.dram_tensor("wp", [16, 128, C], BF16, kind="ExternalInput")
    w1_d = nc.dram_tensor("w1", [NMI, 128, C], BF16, kind="ExternalInput")
    w2_d = nc.dram_tensor("w2", [NMI, 128, C], BF16, kind="ExternalInput")
    wm_d = nc.dram_tensor("wm", [16, 128, IM], BF16, kind="ExternalInput")
    cw_d = nc.dram_tensor("cw", [128, NMQ * DCONV], F32, kind="ExternalInput")
    trig_d = nc.dram_tensor("trig", [128, 1024], BF16, kind="ExternalInput")
    msk_d = nc.dram_tensor("msk", [128, 2048], BF16, kind="ExternalInput")
    rotm_d = nc.dram_tensor("rotm", [128, 128], BF16, kind="ExternalInput")
    out_d = nc.dram_tensor("out", [C, TOK], F32, kind="ExternalOutput")

    dbg = {}
    if DEBUG:
        dbg["sl"] = nc.dram_tensor("d_sl", [NMQ * 128, TOK], BF16, kind="ExternalOutput")
        dbg["x2"] = nc.dram_tensor("d_x2", [C, TOK], F32, kind="ExternalOutput")

    with tile.TileContext(nc) as tc:
        with tc.tile_pool(name="dram", bufs=1, space="DRAM") as dram, \
             tc.tile_pool(name="pers", bufs=1) as pers:
            t1i_kv = dram.tile([2048, 512], BF16)
            t1o_kv = dram.tile([2048, 512], BF16)
            t1i_qa = dram.tile([1024, 512], BF16)
            t1o_qa = dram.tile([1024, 512], BF16)
            t1i_qb = dram.tile([1024, 512], BF16)
            t1o_qb = dram.tile([1024, 512], BF16)
            t2i_a = dram.tile([1024, 512], BF16)
            t2o_a = dram.tile([1024, 512], BF16)
            t2i_b = dram.tile([1024, 512], BF16)
            t2o_b = dram.tile([1024, 512], BF16)

            # ---- constants ----
            cw_sb = pers.tile([128, NMQ * DCONV], F32, tag="cw", name="cw")
            trig_sb = pers.tile([128, 1024], BF16, tag="trig", name="trig")
            msk_sb = pers.tile([128, 2048], BF16, tag="msk", name="msk")
            rotm = pers.tile([128, 128], BF16, tag="rotm", name="rotm")
            nc.sync.dma_start(cw_sb[:], cw_d[:])
            nc.sync.dma_start(trig_sb[:], trig_d[:])
            nc.sync.dma_start(msk_sb[:], msk_d[:])
            nc.sync.dma_start(rotm[:], rotm_d[:])

            ones128 = pers.tile([128, 128], BF16, tag="ones128", name="ones128")
            eps1 = pers.tile([128, 1], F32, tag="eps1", name="eps1")
            nc.gpsimd.memset(ones128[:], 1.0)
            nc.gpsimd.memset(eps1[:], EPS)

            # ---- persistent activations ----
            xh = [pers.tile([128, XW], F32, tag=f"xh{i}", name=f"xh{i}") for i in range(NKC)]
            for i in range(NKC):
                nc.sync.dma_start(xh[i][:], x_d[i * 128:(i + 1) * 128, :])

            # ============================================================
            # Phase A: norm1 -> qkv -> conv -> silu -> rope -> pack A2A1
            # ============================================================
            with tc.tile_pool(name="pa_sb", bufs=1) as pa, \
                 tc.tile_pool(name="pa_ps", bufs=1, space="PSUM") as pap:
                n1 = [pa.tile([128, 2, CH], BF16, tag=f"n1_{i}", name=f"n1_{i}")
                      for i in range(NKC)]
                # --- rmsnorm1: squares (ACT), ones-matmul reduce, full-
                # partition sqrt + reciprocal (no broadcast needed) ---
                ss_ps = pap.tile([128, 1024], F32, tag="big", bufs=3, name="ss")
                ssv = ss_ps[:].rearrange("p (ch c) -> p ch c", ch=2)
                for kk in range(NKC):
                    t = pa.tile([128, 2, CH], BF16, tag="xsq", bufs=4, name="xsq")
                    for ch in range(2):
                        # split the squares across ACT and DVE so the norm
                        # prologue drains in half the time
                        if (kk + ch) % 2 == 0:
                            nc.scalar.activation(t[:, ch, :],
                                                 xh[kk][:, ch * 256:ch * 256 + CH],
                                                 AF.Square)
                        else:
                            nc.vector.tensor_mul(t[:, ch, :],
                                                 xh[kk][:, ch * 256:ch * 256 + CH],
                                                 xh[kk][:, ch * 256:ch * 256 + CH])
                    for ch in range(2):
                        nc.tensor.matmul(ssv[:, ch, 0:CH], ones128[:], t[:, ch, :],
                                         start=(kk == 0), stop=(kk == NKC - 1))
                rt = pa.tile([128, 2, CH], F32, tag="rt", bufs=1, name="rt")
                for ch in range(2):
                    nc.scalar.activation(rt[:, ch, :], ssv[:, ch, 0:CH], AF.Sqrt,
                                         bias=eps1[:], scale=1.0 / C)
                rinv = pa.tile([128, 2, CH], F32, tag="rinv", bufs=1, name="rinv")
                for ch in range(2):
                    nc.vector.reciprocal_approx_fast(rinv[:, ch, :], rt[:, ch, :])
                for kk in range(NKC):
                    for ch in range(2):
                        nc.vector.tensor_mul(n1[kk][:, ch, :],
                                             xh[kk][:, ch * 256:ch * 256 + CH],
                                             rinv[:, ch, :])

                # --- qkv m-tiles: kv first, then even-parity q heads, then
                # odd-parity q heads; A2As fire as each group completes ---
                m_order = [g * 6 + sl for g in range(NG) for sl in (4, 5)] + \
                          [g * 6 + sl for sl in (0, 2) for g in range(NG)] + \
                          [g * 6 + sl for sl in (1, 3) for g in range(NG)]
                for mi_, m in enumerate(m_order):
                    g, slot = m // 6, m % 6
                    wq_sb = pa.tile([128, C], BF16, tag="wq", bufs=3, name="wq")
                    nc.sync.dma_start(wq_sb[:], wq_d[m])
                    big = pap.tile([128, 1024], F32, tag="big", bufs=3, name="big")
                    for kk in range(NKC):
                        for ch in range(2):
                            nc.tensor.matmul(
                                big[:, ch * 512:ch * 512 + CH],
                                wq_sb[:, kk * 128:(kk + 1) * 128],
                                n1[kk][:, ch, :],
                                start=(kk == 0), stop=(kk == NKC - 1))
                    # stitch the two halo-overlapped chunks into one
                    # contiguous 515-wide bf16 strip for the conv taps
                    pre = pa.tile([128, XW], BF16, tag="pre", bufs=3, name="pre")
                    nc.scalar.copy(pre[:, 0:CH], big[:, 0:CH])
                    nc.scalar.copy(pre[:, CH:XW], big[:, 512 + HALO:512 + CH])
                    acc = pa.tile([128, 512], F32, tag="acc", bufs=2, name="acc")
                    nc.vector.tensor_scalar_mul(acc[:], pre[:, 0:512],
                                                cw_sb[:, m * 4:m * 4 + 1])
                    for j in range(1, DCONV):
                        nc.vector.scalar_tensor_tensor(
                            acc[:], pre[:, j:j + 512],
                            cw_sb[:, m * 4 + j:m * 4 + j + 1], acc[:],
                            op0=ALU.mult, op1=ALU.add)
                    sl = pa.tile([128, 512], BF16, tag="sl", bufs=3, name="sl")
                    nc.scalar.activation(sl[:], acc[:], AF.Silu)
                    if DEBUG:
                        nc.sync.dma_start(dbg["sl"][m * 128:(m + 1) * 128, :], sl[:])

                    if slot <= 4:  # q heads and k: rope
                        # rot = [-x2; x1] via PE rotation matmul, then
                        # ro = sl*[c;c] + rot*[s;s]
                        rot_ps = pap.tile([128, 512], F32, tag="rot", bufs=2, name="rot")
                        nc.tensor.matmul(rot_ps[:], rotm[:], sl[:],
                                         start=True, stop=True)
                        tt1 = pa.tile([128, 512], BF16, tag="tt1", bufs=2, name="tt1")
                        nc.vector.tensor_mul(tt1[:], sl[:], trig_sb[:, 0:512])
                        tt2 = pa.tile([128, 512], BF16, tag="tt2", bufs=2, name="tt2")
                        nc.vector.tensor_mul(tt2[:], rot_ps[:], trig_sb[:, 512:1024])
                        ro = pa.tile([128, 512], BF16, tag="ro", bufs=3, name="ro")
                        nc.vector.tensor_add(ro[:], tt1[:], tt2[:])
                        if slot < 4:
                            h = g * QPK + slot
                            dst = t1i_qa if slot % 2 == 0 else t1i_qb
                            d = h // 2
                            nc.sync.dma_start(dst[d * 128:(d + 1) * 128, :], ro[:])
                        else:  # k -> both consumer cores
                            for d in (2 * g, 2 * g + 1):
                                nc.sync.dma_start(
                                    t1i_kv[d * 256:d * 256 + 128, :], ro[:])
                    else:  # v: transpose to token-major (DMA xbar transpose)
                        vtb = pa.tile([128, 512], BF16, tag="vtb", bufs=2, name="vtb")
                        for i in range(4):
                            nc.sync.dma_start_transpose(
                                vtb[:, i * 128:(i + 1) * 128],
                                sl[:, i * 128:(i + 1) * 128])
                        for d in (2 * g, 2 * g + 1):
                            nc.sync.dma_start(
                                t1i_kv[d * 256 + 128:d * 256 + 256, :], vtb[:])
                    if mi_ == 7:  # all kv tiles written -> fire kv exchange
                        nc.gpsimd.collective_compute(
                            "AllToAll", ALU.bypass,
                            replica_groups=[list(range(NCORES))],
                            ins=[t1i_kv[:].opt()], outs=[t1o_kv[:].opt()])
                    if mi_ == 15:  # even-parity q heads -> first q exchange
                        nc.gpsimd.collective_compute(
                            "AllToAll", ALU.bypass,
                            replica_groups=[list(range(NCORES))],
                            ins=[t1i_qa[:].opt()], outs=[t1o_qa[:].opt()])

            nc.gpsimd.collective_compute(
                "AllToAll", ALU.bypass,
                replica_groups=[list(range(NCORES))],
                ins=[t1i_qb[:].opt()], outs=[t1o_qb[:].opt()])

            # ============================================================
            # Phase B: head-parallel causal attention (2 heads per core)
            # + B->C overlap: proj even-k half during the second y A2A
            # ============================================================
            with tc.tile_pool(name="bc_sb", bufs=1) as bc:
                # separate tiles per head-parity so proj's even half has no
                # (false) dependency on the second y exchange
                yk_ev = bc.tile([128, 8, 512], BF16, tag="yk_ev", name="yk_ev")
                yk_od = bc.tile([128, 8, 512], BF16, tag="yk_od", name="yk_od")
                x2 = [bc.tile([128, TOK], F32, tag=f"x2_{i}", name=f"x2_{i}")
                      for i in range(NKC)]
                kv_src = t1o_kv[:].rearrange("(j r) c -> r j c", r=256)
                with tc.tile_pool(name="pb_sb", bufs=1) as pb, \
                     tc.tile_pool(name="pb_ps", bufs=1, space="PSUM") as pbp:
                    y_t = [pb.tile([128, B * T], BF16, tag=f"y{i}", name=f"y{i}")
                           for i in range(2)]
                    for hl in range(2):
                        qsrcT = (t1o_qa if hl == 0 else t1o_qb)
                        q_src = qsrcT[:].rearrange("(j r) c -> r j c", r=128)
                        for beta in range(B):
                            kall = pb.tile([128, 2048], BF16, tag="kall", bufs=2, name="kall")
                            nc.sync.dma_start(
                                kall[:].rearrange("p (j c) -> p j c", c=512),
                                kv_src[0:128, beta * 4:beta * 4 + 4, :])
                            vall = pb.tile([128, 16, 128], BF16, tag="vall", bufs=2, name="vall")
                            nc.sync.dma_start(
                                vall[:].rearrange("p (j i) h -> p j (i h)", j=4),
                                kv_src[128:256, beta * 4:beta * 4 + 4, :])
                            qall = pb.tile([128, 2048], BF16, tag="qall", bufs=2, name="qall")
                            nc.sync.dma_start(
                                qall[:].rearrange("p (j c) -> p j c", c=512),
                                q_src[:, beta * 4:beta * 4 + 4, :])
                            for bp in range(4):
                                o_ps = pbp.tile([128, 512], F32, tag="o", bufs=2, name="o")
                                rs_ps = pbp.tile([128, 512], F32, tag="rs", bufs=2, name="rs")
                                nkb = 2 * bp + 2
                                for kb in range(nkb):
                                    s_ps = pbp.tile([128, 2, 512], F32, tag="s", bufs=2, name="s")
                                    p_sb = pb.tile([128, 2, 512], BF16, tag="p", bufs=4, name="p")
                                    # column offsets: skip fully-masked tq
                                    # ranges in the two diagonal key blocks
                                    if kb == nkb - 2:
                                        c0s, mof = (0, 128), 0
                                    elif kb == nkb - 1:
                                        c0s, mof = (256, 384), 1024
                                    else:
                                        c0s, mof = (0, 0), None
                                    for i in range(2):
                                        c0 = c0s[i]
                                        nc.tensor.matmul(
                                            s_ps[:, i, c0:],
                                            kall[:, kb * 256 + i * 128:kb * 256 + (i + 1) * 128],
                                            qall[:, bp * 512 + c0:(bp + 1) * 512],
                                            start=True, stop=True)
                                    if mof is None:
                                        nc.scalar.activation(p_sb[:], s_ps[:], AF.Exp,
                                                             scale=SCALE)
                                    else:
                                        for i in range(2):
                                            c0 = c0s[i]
                                            nc.scalar.activation(
                                                p_sb[:, i, c0:], s_ps[:, i, c0:],
                                                AF.Exp, scale=SCALE)
                                            nc.vector.tensor_mul(
                                                p_sb[:, i, c0:], p_sb[:, i, c0:],
                                                msk_sb[:, mof + i * 512 + c0:
                                                       mof + (i + 1) * 512])
                                    for i in range(2):
                                        c0 = c0s[i]
                                        nc.tensor.matmul(
                                            o_ps[:, c0:], vall[:, kb * 2 + i, :],
                                            p_sb[:, i, c0:],
                                            start=(kb == 0 and i == 0),
                                            stop=(kb == nkb - 1 and i == 1))
                                    for i in range(2):
                                        c0 = c0s[i]
                                        nc.tensor.matmul(
                                            rs_ps[:, c0:], ones128[:],
                                            p_sb[:, i, c0:],
                                            start=(kb == 0 and i == 0),
                                            stop=(kb == nkb - 1 and i == 1))
                                # the ones-lhsT matmul replicated the softmax
                                # denominator to every partition: reciprocal +
                                # multiply directly, no gather/broadcast.
                                recip = pb.tile([128, 512], F32, tag="recip", bufs=2, name="recip")
                                nc.vector.reciprocal_approx_fast(recip[:], rs_ps[:])
                                nc.vector.tensor_mul(
                                    y_t[hl][:, beta * 2048 + bp * 512:
                                            beta * 2048 + (bp + 1) * 512],
                                    o_ps[:], recip[:])
                        # this head-half is complete: exchange it while the
                        # other half computes
                        t2ih = t2i_a if hl == 0 else t2i_b
                        t2oh = t2o_a if hl == 0 else t2o_b
                        nc.sync.dma_start(
                            t2ih[:].rearrange("(j r) c -> r j c", r=128),
                            y_t[hl][:].rearrange("p (j c) -> p j c", c=512))
                        nc.gpsimd.collective_compute(
                            "AllToAll", ALU.bypass,
                            replica_groups=[list(range(NCORES))],
                            ins=[t2ih[:].opt()], outs=[t2oh[:].opt()])
                        nc.sync.dma_start(
                            (yk_ev if hl == 0 else yk_od)[:],
                            t2oh[:].rearrange("(j r) c -> r j c", r=128))

                    # ---- proj even-k half: only needs t2o_a; fills the PE
                    # while the second y exchange is in flight ----
                    with tc.tile_pool(name="pj_sb", bufs=1) as pj:
                        for mo in range(16):
                            wp_sb = pj.tile([128, C], BF16, tag="wpst", bufs=4, name="wpst")
                            nc.sync.dma_start(wp_sb[:], wp_d[mo])
                            mm_ps = pbp.tile([128, 512], F32, tag="s", bufs=2, name="pmm")
                            for ik in range(8):
                                kk = 2 * ik
                                nc.tensor.matmul(mm_ps[:],
                                                 wp_sb[:, kk * 128:(kk + 1) * 128],
                                                 yk_ev[:, ik, :],
                                                 start=(ik == 0), stop=(ik == 7))
                            nc.vector.tensor_add(x2[mo][:], xh[mo][:, HALO:], mm_ps[:])

                # ---- proj odd-k half + norm2 + MLP ----
                with tc.tile_pool(name="pc_sb", bufs=1) as pc_, \
                     tc.tile_pool(name="pc_ps", bufs=1, space="PSUM") as pcp:
                    for mo in range(16):
                        wp_sb = pc_.tile([128, C], BF16, tag="wst", bufs=4, name="wst")
                        nc.sync.dma_start(wp_sb[:], wp_d[mo])
                        mm_ps = pcp.tile([128, 512], F32, tag="mm", bufs=6, name="mm")
                        for ik in range(8):
                            kk = 2 * ik + 1
                            nc.tensor.matmul(mm_ps[:],
                                             wp_sb[:, kk * 128:(kk + 1) * 128],
                                             yk_od[:, ik, :],
                                             start=(ik == 0), stop=(ik == 7))
                        nc.vector.tensor_add(x2[mo][:], x2[mo][:], mm_ps[:])
                        if DEBUG:
                            nc.sync.dma_start(dbg["x2"][mo * 128:(mo + 1) * 128, :],
                                              x2[mo][:])

                    n2 = [pc_.tile([128, TOK], BF16, tag=f"n2_{i}", name=f"n2_{i}")
                          for i in range(NKC)]
                    h_t = [pc_.tile([128, TOK], BF16, tag=f"h{i}", name=f"h{i}")
                           for i in range(NMI)]
                    ss2 = pcp.tile([128, TOK], F32, tag="nrm", bufs=1, name="nrm")
                    for kk in range(NKC):
                        t = pc_.tile([128, TOK], BF16, tag="x2sq", bufs=4, name="x2sq")
                        nc.scalar.activation(t[:], x2[kk][:], AF.Square)
                        nc.tensor.matmul(ss2[:], ones128[:], t[:],
                                         start=(kk == 0), stop=(kk == NKC - 1))
                    rt2 = pc_.tile([128, TOK], F32, tag="rt2", bufs=1, name="rt2")
                    nc.scalar.activation(rt2[:], ss2[:], AF.Sqrt, bias=eps1[:],
                                         scale=1.0 / C)
                    rinv2 = pc_.tile([128, TOK], F32, tag="rinv2", bufs=1, name="rinv2")
                    nc.vector.reciprocal_approx_fast(rinv2[:], rt2[:])
                    for kk in range(NKC):
                        nc.vector.tensor_mul(n2[kk][:], x2[kk][:], rinv2[:])

                    for mi in range(NMI):
                        w1_sb = pc_.tile([128, C], BF16, tag="wst", bufs=4, name="wst")
                        nc.sync.dma_start(w1_sb[:], w1_d[mi])
                        h1_ps = pcp.tile([128, TOK], F32, tag="mm", bufs=6, name="mm")
                        for kk in range(NKC):
                            nc.tensor.matmul(h1_ps[:],
                                             w1_sb[:, kk * 128:(kk + 1) * 128],
                                             n2[kk][:],
                                             start=(kk == 0), stop=(kk == NKC - 1))
                        s1 = pc_.tile([128, TOK], BF16, tag="s1", bufs=2, name="s1")
                        nc.scalar.activation(s1[:], h1_ps[:], AF.Silu)
                        w2_sb = pc_.tile([128, C], BF16, tag="wst", bufs=4, name="wst")
                        nc.sync.dma_start(w2_sb[:], w2_d[mi])
                        h2_ps = pcp.tile([128, TOK], F32, tag="mm", bufs=6, name="mm")
                        for kk in range(NKC):
                            nc.tensor.matmul(h2_ps[:],
                                             w2_sb[:, kk * 128:(kk + 1) * 128],
                                             n2[kk][:],
                                             start=(kk == 0), stop=(kk == NKC - 1))
                        nc.vector.tensor_mul(h_t[mi][:], s1[:], h2_ps[:])

                    with tc.tile_pool(name="pcm", bufs=1) as pcm:
                        for mo in range(16):
                            wm_sb = pcm.tile([128, IM], BF16, tag="wm", bufs=2, name="wm")
                            nc.sync.dma_start(wm_sb[:], wm_d[mo])
                            mp_ps = pcp.tile([128, TOK], F32, tag="mm", bufs=6, name="mm")
                            for ki in range(NMI):
                                nc.tensor.matmul(mp_ps[:],
                                                 wm_sb[:, ki * 128:(ki + 1) * 128],
                                                 h_t[ki][:],
                                                 start=(ki == 0), stop=(ki == NMI - 1))
                            outsb = pc_.tile([128, TOK], F32, tag="outsb", bufs=2, name="outsb")
                            nc.vector.tensor_add(outsb[:], x2[mo][:], mp_ps[:])
                            nc.sync.dma_start(out_d[mo * 128:(mo + 1) * 128, :], outsb[:])

    nc.compile()
    return nc


# --------------------------------------------------------------------------
# host-side prep / gather
# --------------------------------------------------------------------------

def _prep_lhsT(w, nm, nk):
    """w: (out, in) f32 -> (nm, 128, nk*128) bf16 where
    prep[m][p][k*128+c] = w[m*128+c, k*128+p]."""
    o, i = w.shape
    assert o == nm * 128 and i == nk * 128
    r = w.reshape(nm, 128, nk, 128).transpose(0, 3, 2, 1)  # (m, p, k, c)
    return np.ascontiguousarray(r.reshape(nm, 128, nk * 128)).astype(ml_dtypes.bfloat16)


def _host_inputs(inputs):
    x = np.asarray(inputs["x"], np.float32)          # (B, T, C)
    cos = np.asarray(inputs["cos"], np.float32)      # (T, 64)
    sin = np.asarray(inputs["sin"], np.float32)
    n1w = np.asarray(inputs["norm1_w"], np.float32)
    n2w = np.asarray(inputs["norm2_w"], np.float32)

    # fold rmsnorm weights into the (pre-transposed) weight matrices
    attn_w = np.asarray(inputs["attn_w"], np.float32) * n1w[None, :]
    fc1_w = np.asarray(inputs["fc1_w"], np.float32) * n2w[None, :]
    fc2_w = np.asarray(inputs["fc2_w"], np.float32) * n2w[None, :]
    proj_w = np.asarray(inputs["proj_w"], np.float32)
    mlp_w = np.asarray(inputs["mlp_proj_w"], np.float32)

    wq = _prep_lhsT(attn_w, NMQ, NKC)
    wp = _prep_lhsT(proj_w, 16, NKC)
    w1 = _prep_lhsT(fc1_w, NMI, NKC)
    w2 = _prep_lhsT(fc2_w, NMI, NKC)
    wm = _prep_lhsT(mlp_w, 16, NMI)

    # conv weights in qkv m-tile order: per g: q0..q3 (qconv), k, v
    cw = np.zeros((NMQ, 128, DCONV), np.float32)
    qc = np.asarray(inputs["qconv_w"], np.float32)
    kc = np.asarray(inputs["kconv_w"], np.float32)
    vc = np.asarray(inputs["vconv_w"], np.float32)
    for g in range(NG):
        for s in range(QPK):
            cw[g * 6 + s] = qc[(g * QPK + s) * 128:(g * QPK + s + 1) * 128]
        cw[g * 6 + 4] = kc[g * 128:(g + 1) * 128]
        cw[g * 6 + 5] = vc[g * 128:(g + 1) * 128]
    cw = np.ascontiguousarray(cw.transpose(1, 0, 2).reshape(128, NMQ * DCONV))

    # paired-block diag masks, each (128, 2, 512) flattened to (128, 1024):
    # mskA: kb == nkb-2 (tk rel = i*128+p); mskB: kb == nkb-1 (tk rel = 256+i*128+p)
    p = np.arange(128)[:, None]
    f = np.arange(512)[None, :]
    mskA = np.concatenate([(p <= f), (p + 128 <= f)], axis=1)
    mskB = np.concatenate([(p + 256 <= f), (p + 384 <= f)], axis=1)
    msk = np.concatenate([mskA, mskB], axis=1).astype(np.float32)
    msk = msk.astype(ml_dtypes.bfloat16)

    # rope rotation: rot = rotm.T @ x = [-x2; x1]
    rotm = np.zeros((128, 128), np.float32)
    for m in range(64):
        rotm[m + 64, m] = -1.0
        rotm[m, m + 64] = 1.0
    rotm = rotm.astype(ml_dtypes.bfloat16)

    # per-core x (feature-major with halo) and trig
    xt = x.transpose(0, 2, 1)                        # (B, C, T)
    xpad = np.concatenate([np.zeros((B, C, HALO), np.float32), xt], axis=2)
    cosT = cos.T                                     # (64, T)
    sinT = sin.T
    in_maps = []
    for c in range(NCORES):
        beta, tb = c // 4, (512 * c) % 2048
        xc = np.ascontiguousarray(xpad[beta, :, tb:tb + XW])
        cs = np.concatenate([cosT[:, tb:tb + TOK], cosT[:, tb:tb + TOK]], axis=0)
        ss = np.concatenate([sinT[:, tb:tb + TOK], sinT[:, tb:tb + TOK]], axis=0)
        trig = np.concatenate([cs, ss], axis=1).astype(ml_dtypes.bfloat16)
        in_maps.append({
            "x": xc, "wq": wq, "wp": wp, "w1": w1, "w2": w2, "wm": wm,
            "cw": cw, "trig": np.ascontiguousarray(trig), "msk": msk,
            "rotm": rotm,
        })
    return in_maps


_NC_CACHE = None


def kernel(**inputs) -> np.ndarray:
    global LAST_RESULTS, _NC_CACHE
    if _NC_CACHE is None:
        _NC_CACHE = build_nc()
    nc = _NC_CACHE
    in_maps = _host_inputs(inputs)
    res = run_bass_kernel_spmd(nc, in_maps, list(range(NCORES)), trace=TRACE)
    LAST_RESULTS = res
    out = np.zeros((B, T, C), np.float32)
    for c in range(NCORES):
        oc = res.results[c]["out"]                   # (C, TOK) feature-major
        beta, tb = c // 4, (512 * c) % 2048
        out[beta, tb:tb + TOK, :] = oc.T
    return out
